# revision 5
# baseline (speedup 1.0000x reference)
"""Causal self-attention (B=4, T=2048, C=1024, H=16, D=64) on 8 TRN2 NeuronCores.

Sharding: batch x head-group. Core c handles batch b = c//2 and heads
hg*8..hg*8+8 where hg = c%2 (data parallel on batch, tensor parallel on heads;
w_qkv column-sharded, w_out row-sharded). Each core is fully independent; the
host sums the two per-batch partial outputs and adds the bias terms.

v3 (engine-rebalanced): bf16 matmuls everywhere (PSUM fp32). Measured TRN2
rates: PE streams 1 bf16 col/cycle @2.4GHz; ScalarE (Act) ~0.88ns/col;
DVE ~0.96ns/col; Pool cannot read PSUM. The baseline was double-walled by
PE (~204us of streamed columns) and Act (~172us of exp). Changes:
  - exp is split between ScalarE (exact activation-exp) and DVE (Schraudolph
    bitcast exp: i16 = trunc(A*s + B) written into the bf16 pt tile via an
    int16 bitcast view; ~3% sawtooth on the affected keys, which largely
    averages out inside softmax). Split fraction tuned so both engines stay
    well under the PE roofline.
  - causal masking of the diagonal S blocks moved off the PE (was: iden/bigu
    matmul accumulating -1e30) to Pool affine_select on the exp'd pt tile
    (fill=0 kills the j>i half). Saves ~16k PE columns; Pool is otherwise idle.
  - sp tiles merge both heads of a pr (one [128, 2*G*512] PSUM tile): one exp
    instruction covers both heads' scores -> fewer fixed per-instruction
    overheads (~220ns each, PSUM access latency).
  - O-normalize reads the PV PSUM directly (reciprocal_approx_fast on the
    denominator column + per-i-tile tensor_scalar_mul), no staging copy.
  - QKV/V/y evacuations are distributed across ScalarE(Copy)/DVE by a static
    deficit balancer; b_qkv==0 (per the problem spec) enables the pure-copy
    fast path, with a general tensor_scalar_add fallback otherwise.
"""

import numpy as np

import concourse.bass as bass
import concourse.bacc as bacc
import concourse.mybir as mybir
from concourse.tile import TileContext

# ---- problem constants (hardcoded per contract) ----
B, T, C = 4, 2048, 1024
H_GLOBAL, D = 16, 64
HL = 8                      # local heads per core
N_CORES = 8
P = 128
KT_C = C // P               # 8 contraction tiles over C
NT = T // P                 # 16 t-tiles
IB = 512                    # query block (i-chunk)
NIC = T // IB               # 4 i-chunks
G = 2                       # j-tiles per exp group
M1 = 3 * HL * D             # 1536 local qkv cols
F32 = mybir.dt.float32
BF16 = mybir.dt.bfloat16
I16 = mybir.dt.int16
SCALE = 1.0 / np.sqrt(D).astype(np.float32)

# Schraudolph constants for bf16 exp(s*SCALE) via int16 bitcast:
# i16 = trunc(A_SCH * s + B_SCH); bits = bf16(exp(s/8) * (1 + ~3% sawtooth)).
LOG2E = float(np.log2(np.e))
A_SCH = 128.0 * LOG2E * float(SCALE)
B_SCH = 127.0 * 128.0 - 5.0

# measured engine rates (ns per 128-partition column) + per-instr overhead ns
RATE = {"act": 0.88, "dve": 0.96}
IOVH = {"act": 220.0, "dve": 200.0}


class Filler:
    """FIFO of instruction-issuing thunks, drained into another loop."""

    def __init__(self):
        self.thunks = []

    def add(self, *fns):
        self.thunks.extend(fns)

    def emit(self, n):
        for _ in range(min(n, len(self.thunks))):
            self.thunks.pop(0)()

    def flush(self):
        while self.thunks:
            self.thunks.pop(0)()


class Balancer:
    """Static deficit scheduler across ScalarE/DVE for exp + evac ops."""

    def __init__(self, preload=None, sch_frac_cap=0.45):
        self.busy = dict(preload or {"act": 0.0, "dve": 0.0})
        self.exp_cols = {"act": 0.0, "dve": 0.0}
        self.cap = sch_frac_cap

    def pick(self, cols, exp=False):
        # projected finish time if assigned
        cost = {e: self.busy[e] + cols * RATE[e] + IOVH[e] for e in RATE}
        if exp:
            tot = self.exp_cols["act"] + self.exp_cols["dve"] + cols
            if self.exp_cols["dve"] + cols > self.cap * tot:
                e = "act"          # accuracy cap on the Schraudolph share
            else:
                e = min(cost, key=cost.get)
            self.exp_cols[e] += cols
        else:
            e = min(cost, key=cost.get)
        self.busy[e] += cols * RATE[e] + IOVH[e]
        return e

    def charge(self, e, cols, n_instr=1):
        self.busy[e] += cols * RATE[e] + n_instr * IOVH[e]


def build_nc(repeat=1, phases="BCD", fake_exp=False, b1_zero=True,
             sch_cap=0.45, verbose=False):
    nc = bacc.Bacc("TRN2", target_bir_lowering=False)
    EXPF = mybir.ActivationFunctionType.Exp
    MUL, ADD = mybir.AluOpType.mult, mybir.AluOpType.add

    xT = nc.dram_tensor("xT", [C, T], BF16, kind="ExternalInput").ap()
    w1 = nc.dram_tensor("w1", [C, M1], BF16, kind="ExternalInput").ap()
    b1 = nc.dram_tensor("b1", [M1], F32, kind="ExternalInput").ap()
    w2 = nc.dram_tensor("w2", [HL * D, C], BF16, kind="ExternalInput").ap()
    y = nc.dram_tensor("y", [T, C], BF16, kind="ExternalOutput").ap()

    xT_r = xT.rearrange("(k p) t -> p k t", p=P)

    with TileContext(nc) as tc:
      for _rep in range(repeat):
        bal = Balancer(sch_frac_cap=sch_cap)

        def copy_psum(dst, src):
            """PSUM->SBUF evacuation on whichever engine is less loaded."""
            e = bal.pick(src.shape[-1])
            if e == "act":
                nc.scalar.copy(dst, src)
            else:
                nc.vector.tensor_copy(out=dst, in_=src)

        with tc.tile_pool(name="persist", bufs=1) as persist, \
             tc.tile_pool(name="xs", bufs=2) as xs, \
             tc.tile_pool(name="fpsum", bufs=2, space="PSUM") as fpsum, \
             tc.tile_pool(name="spsum", bufs=1, space="PSUM") as spool, \
             tc.tile_pool(name="opsum", bufs=2, space="PSUM") as opool, \
             tc.tile_pool(name="ptp", bufs=2) as ptp, \
             tc.tile_pool(name="recp", bufs=2) as recp, \
             tc.tile_pool(name="onrm", bufs=2) as onrm, \
             tc.tile_pool(name="yout", bufs=3) as yout:
            b1_sb = persist.tile([P, 12], F32, tag="b1", name="b1_sb")
            w2_sb = persist.tile([P, HL * D // P, C], BF16, tag="w2", name="w2_sb")
            w1_sb = persist.tile([P, KT_C, M1], BF16, tag="w1", name="w1_sb")

            # per-chunk persistent activations (bf16)
            QTc = [[persist.tile([P, IB], BF16, tag=f"QT{pr}_{tc_}",
                                 name=f"QT{pr}_{tc_}")
                    for tc_ in range(NIC)] for pr in range(HL // 2)]
            KTc = [[persist.tile([P, IB], BF16, tag=f"KT{pr}_{tc_}",
                                 name=f"KT{pr}_{tc_}")
                    for tc_ in range(NIC)] for pr in range(HL // 2)]
            V = [persist.tile([P, HL * (D + 1)], BF16, tag=f"V{tt}",
                              name=f"V{tt}")
                 for tt in range(NT)]
            OPc = [[persist.tile([P, IB], BF16, tag=f"OP{pr}_{tc_}",
                                 name=f"OP{pr}_{tc_}")
                    for tc_ in range(NIC)] for pr in range(HL // 2)]

            # V denominator columns = 1.0, written once on the idle Pool
            for tt in range(NT):
                nc.gpsimd.memset(
                    V[tt].rearrange("p (h x) -> p h x", x=D + 1)[:, :, D:D + 1],
                    1.0)

            # ---------------- phase B thunks (QKV projection chunk) --------
            def b_chunk_thunks(tc_, fill):
                xc_cell = {}

                def dma_xc(tc_=tc_):
                    xc = xs.tile([P, KT_C, IB], BF16, tag="xc", name="xc")
                    nc.sync.dma_start(out=xc[:, 0:4, :],
                                      in_=xT_r[:, 0:4, tc_ * IB:(tc_ + 1) * IB])
                    nc.scalar.dma_start(out=xc[:, 4:8, :],
                                        in_=xT_r[:, 4:8,
                                                 tc_ * IB:(tc_ + 1) * IB])
                    xc_cell["t"] = xc
                fill.add(dma_xc)

                for ttl in range(IB // P):
                    tt = tc_ * (IB // P) + ttl
                    vp_cell = {}
                    for k in range(KT_C):
                        def mm(k=k, ttl=ttl, vp_cell=vp_cell):
                            if k == 0:
                                vp_cell["t"] = fpsum.tile([P, HL * D], F32,
                                                          tag="fp", name="vp")
                            nc.tensor.matmul(
                                vp_cell["t"][:, :],
                                lhsT=xc_cell["t"][:, k, ttl * P:(ttl + 1) * P],
                                rhs=w1_sb[:, k, 2 * HL * D:3 * HL * D],
                                start=(k == 0), stop=(k == KT_C - 1))
                        fill.add(mm)

                    def evac(tt=tt, vp_cell=vp_cell):
                        copy_psum(
                            V[tt].rearrange(
                                "p (h x) -> p h x", x=D + 1)[:, :, 0:D],
                            vp_cell["t"].rearrange("p (h x) -> p h x", x=D))
                    fill.add(evac)

                for pr in range(HL // 2):
                    for qk in range(2):
                        qp_cell = {}
                        for k in range(KT_C):
                            def mm(k=k, pr=pr, qk=qk, qp_cell=qp_cell):
                                if k == 0:
                                    qp_cell["t"] = fpsum.tile(
                                        [P, IB], F32, tag="fp", name="qp")
                                nc.tensor.matmul(
                                    qp_cell["t"][:, :],
                                    lhsT=w1_sb[:, k,
                                               qk * HL * D + pr * P:
                                               qk * HL * D + (pr + 1) * P],
                                    rhs=xc_cell["t"][:, k, :],
                                    start=(k == 0), stop=(k == KT_C - 1))
                            fill.add(mm)

                        def evac(pr=pr, qk=qk, tc_=tc_, qp_cell=qp_cell):
                            dst = (QTc if qk == 0 else KTc)[pr][tc_]
                            if b1_zero:
                                copy_psum(dst[:, :], qp_cell["t"][:, :])
                            else:
                                nc.vector.tensor_scalar_add(
                                    dst[:, :], qp_cell["t"][:, :],
                                    b1_sb[:, 4 * qk + pr:4 * qk + pr + 1])
                        fill.add(evac)

            # ---------------- phase D thunks (out projection chunk) --------
            def d_chunk_thunks(tc_, fill, tail=False):
                for ttl in range(IB // P):
                    for cc in range(C // IB):
                        yp_cell = {}
                        for pr in range(HL // 2):
                            def mm(pr=pr, ttl=ttl, cc=cc, tc_=tc_,
                                   yp_cell=yp_cell):
                                if pr == 0:
                                    yp_cell["t"] = fpsum.tile(
                                        [P, IB], F32, tag="fp", name="yp")
                                nc.tensor.matmul(
                                    yp_cell["t"][:, :],
                                    lhsT=OPc[pr][tc_][:, ttl * P:(ttl + 1) * P],
                                    rhs=w2_sb[:, pr, cc * IB:(cc + 1) * IB],
                                    start=(pr == 0), stop=(pr == HL // 2 - 1))
                            fill.add(mm)

                        def evac(ttl=ttl, cc=cc, tc_=tc_, yp_cell=yp_cell):
                            tt = tc_ * (IB // P) + ttl
                            ysb = yout.tile([P, IB], BF16, tag="ysb",
                                            name="ysb")
                            copy_psum(ysb[:, :], yp_cell["t"][:, :])
                            (nc.scalar if tail else nc.sync).dma_start(
                                out=y[tt * P:(tt + 1) * P,
                                      cc * IB:(cc + 1) * IB],
                                in_=ysb[:, :])
                        fill.add(evac)

            # ---------------- fused pipeline over i-chunks -----------------
            fill = Filler()
            b_chunk_thunks(0, fill)
            fill.emit(1)        # xc(0) DMA first in the in-order SP queue
            nc.sync.dma_start(out=b1_sb[:, :],
                              in_=b1.rearrange("(m p) -> p m", p=P))
            # w1 k-tiles alternate between the two hwdge queues (SP and
            # Activation -- ScalarE is idle through phase B) to halve the
            # prologue weight-load latency
            for k in range(KT_C):
                eng = nc.sync if k % 2 == 0 else nc.scalar
                eng.dma_start(out=w1_sb[:, k, :],
                              in_=w1.rearrange("(k p) m -> p k m",
                                               p=P)[:, k, :])
            nc.scalar.dma_start(out=w2_sb[:, :, :],
                                in_=w2.rearrange("(k p) c -> p k c", p=P))
            fill.flush()

            for ic in range(NIC):
                # queue fillers: xc prefetch first, then previous D chunk
                # interleaved with the next B chunk
                fa, fb = Filler(), Filler()
                if "D" in phases and ic > 0:
                    d_chunk_thunks(ic - 1, fa)
                if ic + 1 < NIC:
                    b_chunk_thunks(ic + 1, fb)
                fb.emit(1)      # xc(ic+1) DMA prefetch
                while fa.thunks or fb.thunks:
                    if fa.thunks:
                        fill.add(fa.thunks.pop(0))
                    if fb.thunks:
                        fill.add(fb.thunks.pop(0))

                if "C" in phases:
                    njt = (ic + 1) * (IB // P)
                    for pr in range(HL // 2):
                        # O accumulators in [i, d|den] layout: per i-tile a
                        # 65-wide block (64 dims + softmax denominator)
                        ops = [opool.tile([P, 4 * (D + 1)], F32, tag="op",
                                          name=f"o{h2}") for h2 in range(2)]
                        onorm = onrm.tile([P, IB], BF16, tag="on",
                                          name="onorm")
                        ngr = (njt + G - 1) // G
                        for g in range(ngr):
                            jts = list(range(g * G, min((g + 1) * G, njt)))
                            nj = len(jts)
                            srels = [max(0, jt * P - ic * IB) for jt in jts]
                            # both heads' scores in one sp tile:
                            # [h2, jl, i] laid out as 2*G blocks of IB cols
                            sp = spool.tile([P, 2 * G * IB], F32, tag="sp",
                                            name="sp")
                            # h2 innermost: consecutive S matmuls alternate
                            # PE row-halves (tile rows 0/64), which overlaps
                            # the K=64 weight loads with streaming
                            for jl, jt in enumerate(jts):
                                srel = srels[jl]
                                for h2 in range(2):
                                    hs = h2 * D
                                    base = h2 * G * IB
                                    nc.tensor.matmul(
                                        sp[:, base + jl * IB + srel:
                                           base + (jl + 1) * IB],
                                        lhsT=KTc[pr][jt // 4][
                                            hs:hs + D,
                                            (jt % 4) * P:(jt % 4 + 1) * P],
                                        rhs=QTc[pr][ic][
                                            hs:hs + D, srel:IB],
                                        start=True, stop=True)
                            pt = ptp.tile([P, 2 * G * IB], BF16, tag="pt",
                                          name="pt")

                            def do_exp(c0, c1):
                                cols = c1 - c0
                                if fake_exp:
                                    nc.vector.tensor_copy(
                                        out=pt[:, c0:c1], in_=sp[:, c0:c1])
                                    return
                                e = bal.pick(cols, exp=True)
                                if e == "act":
                                    nc.scalar.activation(
                                        pt[:, c0:c1], sp[:, c0:c1],
                                        EXPF, scale=float(SCALE))
                                else:
                                    nc.vector.tensor_scalar(
                                        out=pt[:, c0:c1].bitcast(I16),
                                        in0=sp[:, c0:c1],
                                        scalar1=A_SCH, scalar2=B_SCH,
                                        op0=MUL, op1=ADD)

                            if not any(srels):
                                # both heads' blocks are contiguous
                                do_exp(0, 2 * G * IB)
                            else:
                                for h2 in range(2):
                                    base = h2 * G * IB
                                    for jl in range(nj):
                                        do_exp(base + jl * IB + srels[jl],
                                               base + (jl + 1) * IB)
                            # causal mask for diagonal blocks: zero j>i on the
                            # exp'd tile (Pool affine_select, SBUF-only)
                            for h2 in range(2):
                                base = h2 * G * IB
                                for jl, jt in enumerate(jts):
                                    rel = jt * P - ic * IB
                                    if rel >= 0:
                                        nc.gpsimd.affine_select(
                                            out=pt[:, base + jl * IB + rel:
                                                   base + jl * IB + rel + P],
                                            in_=pt[:, base + jl * IB + rel:
                                                   base + jl * IB + rel + P],
                                            pattern=[[1, P]],
                                            compare_op=mybir.AluOpType.is_ge,
                                            fill=0.0,
                                            base=0, channel_multiplier=-1)
                            for h2 in range(2):
                                h = pr * 2 + h2
                                base = h2 * G * IB
                                for jl, jt in enumerate(jts):
                                    for it in range(IB // P):
                                        git = ic * (IB // P) + it
                                        if jt > git:
                                            continue
                                        nc.tensor.matmul(
                                            ops[h2][:, it * (D + 1):
                                                    (it + 1) * (D + 1)],
                                            lhsT=pt[:, base + jl * IB + it * P:
                                                    base + jl * IB +
                                                    (it + 1) * P],
                                            rhs=V[jt][:, h * (D + 1):
                                                      (h + 1) * (D + 1)],
                                            start=(jt == 0 and it == 0),
                                            stop=(jt == njt - 1 and it == 3))
                                fill.emit(2)
                        # normalize per i-partition: O[i, d] * recip(den[i])
                        # directly from PSUM, then DMA-xbar transpose of
                        # [128 i, 128 d] blocks into the O^T [d, t] layout.
                        for h2 in range(2):
                            rcol = recp.tile([P, 4], F32, tag="rc",
                                             name="rcol")
                            den = ops[h2].rearrange("p (i x) -> p i x",
                                                    x=D + 1)[:, :, D:D + 1]
                            nc.vector.reciprocal_approx_fast(
                                out=rcol.rearrange("p (i x) -> p i x",
                                                   x=1)[:, :, :],
                                in_=den)
                            for it in range(IB // P):
                                nc.vector.tensor_scalar_mul(
                                    onorm[:, it * P + h2 * D:
                                          it * P + (h2 + 1) * D],
                                    ops[h2][:, it * (D + 1):it * (D + 1) + D],
                                    rcol[:, it:it + 1])
                            bal.charge("dve", 4 + 4 * D, n_instr=5)
                            fill.emit(2)
                        for it in range(IB // P):
                            nc.sync.dma_start_transpose(
                                out=OPc[pr][ic][:, it * P:(it + 1) * P],
                                in_=onorm[:, it * P:(it + 1) * P])
                        fill.emit(2)
                fill.flush()

            if "D" in phases:
                d_chunk_thunks(NIC - 1, fill, tail=True)
                fill.flush()
        if verbose:
            print(f"[build] balancer busy(ns): "
                  f"act={bal.busy['act']:.0f} dve={bal.busy['dve']:.0f} "
                  f"exp cols: act={bal.exp_cols['act']:.0f} "
                  f"dve={bal.exp_cols['dve']:.0f}")
    nc.compile()
    return nc


_NC_CACHE = {}


def _get_nc(key=()):
    if key not in _NC_CACHE:
        _NC_CACHE[key] = build_nc()
    return _NC_CACHE[key]


def shard_inputs(x, w_qkv, b_qkv, w_out):
    """Build the 8 per-core input maps (bf16 matmul inputs)."""
    import ml_dtypes
    bf = ml_dtypes.bfloat16
    x = np.asarray(x, dtype=np.float32)
    w_qkv = np.asarray(w_qkv, dtype=np.float32)
    b_qkv = np.asarray(b_qkv, dtype=np.float32)
    w_out = np.asarray(w_out, dtype=np.float32)
    in_maps = []
    for core in range(N_CORES):
        b, hg = core // 2, core % 2
        cs = hg * HL * D              # 512-wide contiguous head-group slice
        w1 = np.ascontiguousarray(np.concatenate(
            [w_qkv[:, cs:cs + HL * D],
             w_qkv[:, C + cs:C + cs + HL * D],
             w_qkv[:, 2 * C + cs:2 * C + cs + HL * D]], axis=1))
        b1 = np.ascontiguousarray(np.concatenate(
            [b_qkv[cs:cs + HL * D],
             b_qkv[C + cs:C + cs + HL * D],
             b_qkv[2 * C + cs:2 * C + cs + HL * D]]))
        in_maps.append({
            "xT": np.ascontiguousarray(x[b].T).astype(bf),
            "w1": w1.astype(bf),
            "b1": b1,
            "w2": np.ascontiguousarray(w_out[cs:cs + HL * D, :]).astype(bf),
        })
    return in_maps


def combine_outputs(results, b_qkv, w_out, b_out):
    """Sum per-batch partials from the two head-group cores + bias terms."""
    bias_vec = (np.asarray(b_qkv[2 * C:3 * C], dtype=np.float32) @
                np.asarray(w_out, dtype=np.float32) +
                np.asarray(b_out, dtype=np.float32))
    y = np.empty((B, T, C), dtype=np.float32)
    for b in range(B):
        y[b] = (np.asarray(results[2 * b]["y"], dtype=np.float32) +
                np.asarray(results[2 * b + 1]["y"], dtype=np.float32) +
                bias_vec)
    return y


def kernel(x, w_qkv, b_qkv, w_out, b_out, *, trace=False, _sink=None):
    from concourse.bass_utils import run_bass_kernel_spmd
    b1z = not np.any(np.asarray(b_qkv))
    nc = (_get_nc() if b1z
          else _NC_CACHE.setdefault("b1", build_nc(b1_zero=False)))
    in_maps = shard_inputs(x, w_qkv, b_qkv, w_out)
    res = run_bass_kernel_spmd(nc, in_maps, core_ids=list(range(N_CORES)),
                               trace=trace)
    if _sink is not None:
        _sink.append(res)
    return combine_outputs(res.results, b_qkv, w_out, b_out)


# revision 24
# speedup vs baseline: 1.6160x; 1.6160x over previous
"""Causal self-attention (B=4, T=2048, C=1024, H=16, D=64) on 8 TRN2 NeuronCores.

Sharding: batch x head-group. Core c handles batch b = c//2 and heads
hg*8..hg*8+8 where hg = c%2 (data parallel on batch, tensor parallel on heads;
w_qkv column-sharded, w_out row-sharded). Each core is fully independent; the
host sums the two per-batch partial outputs and adds the bias terms.

v9 (engine-rebalanced + PE tile concurrency + deep software pipeline). Measured TRN2 rates: a K=128
bf16 matmul streams ~240ns/512 cols; two K=64 matmuls on opposite PE row
halves run CONCURRENTLY (~105ns/512 cols each); ScalarE exp ~0.88ns/col +
~220ns/instr; DVE tensor_scalar from PSUM ~1.33ns/col effective; Pool cannot
read PSUM. Changes vs the original baseline:
  - S^T matmuls (K=64) alternate the two heads of a pr (PE rows 0-63 / 64-127)
    instruction-by-instruction -> 2x tile concurrency on the attention scores.
  - B/D projection chains stay K=128 (mixed tile rows inside one PSUM
    accumulation group crash the device, so the K=64 trick is only legal for
    the single-matmul S groups).
  - exp is split between ScalarE (exact activation-exp) and DVE (Schraudolph
    bitcast exp: i16 = trunc(A*s + B) into the bf16 pt tile via an int16
    bitcast view; ~3% sawtooth that self-cancels inside softmax).
  - phase C is software-pipelined at G=1 with a 4-slot sp ring (one PSUM
    bank per score tile) and PV lagging 3 groups: the PE streams scores two
    groups ahead of the exp being consumed, hiding the exp queue latency
    that otherwise serializes ~85us of the attention inner loop.
  - causal masking of diagonal S blocks moved off the PE (was iden/bigu
    matmuls) to Pool affine_select on the exp'd pt tile (fill=0).
  - O-normalize reads the PV PSUM directly (reciprocal_approx_fast +
    tensor_scalar_mul), no staging copy.
  - QKV/V/y evacuations distributed across ScalarE(Copy)/DVE by a deficit
    balancer; b_qkv==0 (per spec) enables the pure-copy fast path (general
    tensor_scalar_add fallback otherwise).
"""

import numpy as np

import concourse.bass as bass
import concourse.bacc as bacc
import concourse.mybir as mybir
from concourse.tile import TileContext

# ---- problem constants (hardcoded per contract) ----
B, T, C = 4, 2048, 1024
H_GLOBAL, D = 16, 64
HL = 8                      # local heads per core
N_CORES = 8
P = 128
KT_C = C // P               # 8 contraction tiles over C
NT = T // P                 # 16 t-tiles
IB = 512                    # query block (i-chunk)
NIC = T // IB               # 4 i-chunks
G = 1                       # j-tiles per exp group
M1 = 3 * HL * D             # 1536 local qkv cols
F32 = mybir.dt.float32
BF16 = mybir.dt.bfloat16
I16 = mybir.dt.int16
SCALE = 1.0 / np.sqrt(D).astype(np.float32)

# Schraudolph constants for bf16 exp(s*SCALE) via int16 bitcast:
# i16 = trunc(A_SCH * s + B_SCH); bits = bf16(exp(s/8) * (1 + ~3% sawtooth)).
LOG2E = float(np.log2(np.e))
A_SCH = 128.0 * LOG2E * float(SCALE)
B_SCH = 127.0 * 128.0 - 5.0

# measured engine rates (ns per 128-partition column) + per-instr overhead ns
RATE = {"act": 0.88, "dve": 0.96}
IOVH = {"act": 220.0, "dve": 200.0}


class Filler:
    """FIFO of instruction-issuing thunks, drained into another loop."""

    def __init__(self):
        self.thunks = []

    def add(self, *fns):
        self.thunks.extend(fns)

    def emit(self, n):
        for _ in range(min(n, len(self.thunks))):
            self.thunks.pop(0)()

    def flush(self):
        while self.thunks:
            self.thunks.pop(0)()


class Balancer:
    """Static deficit scheduler across ScalarE/DVE for exp + evac ops."""

    def __init__(self, sch_frac_cap=0.45):
        self.busy = {"act": 0.0, "dve": 0.0}
        self.exp_cols = {"act": 0.0, "dve": 0.0}
        self.cap = sch_frac_cap

    def pick(self, cols, exp=False):
        cost = {e: self.busy[e] + cols * RATE[e] + IOVH[e] for e in RATE}
        if not exp:
            # keep ScalarE's queue exp-dominated: copies prefer DVE unless
            # DVE has fallen well behind
            cost["act"] += 15000.0
        if exp:
            tot = self.exp_cols["act"] + self.exp_cols["dve"] + cols
            if self.exp_cols["dve"] + cols > self.cap * tot:
                e = "act"          # accuracy cap on the Schraudolph share
            else:
                e = min(cost, key=cost.get)
            self.exp_cols[e] += cols
        else:
            e = min(cost, key=cost.get)
        self.busy[e] += cols * RATE[e] + IOVH[e]
        return e

    def charge(self, e, cols, n_instr=1):
        self.busy[e] += cols * RATE[e] + n_instr * IOVH[e]


def build_nc(repeat=1, phases="BCD", fake_exp=False, b1_zero=True,
             sch_cap=0.45, verbose=False):
    nc = bacc.Bacc("TRN2", target_bir_lowering=False)
    EXPF = mybir.ActivationFunctionType.Exp
    MUL, ADD = mybir.AluOpType.mult, mybir.AluOpType.add

    xT = nc.dram_tensor("xT", [C, T], BF16, kind="ExternalInput").ap()
    w1 = nc.dram_tensor("w1", [C, M1], BF16, kind="ExternalInput").ap()
    b1 = nc.dram_tensor("b1", [M1], F32, kind="ExternalInput").ap()
    w2 = nc.dram_tensor("w2", [HL * D, C], BF16, kind="ExternalInput").ap()
    y = nc.dram_tensor("y", [T, C], BF16, kind="ExternalOutput").ap()

    xT_r = xT.rearrange("(k p) t -> p k t", p=P)

    with TileContext(nc) as tc:
      for _rep in range(repeat):
        bal = Balancer(sch_frac_cap=sch_cap)

        def copy_psum(dst, src):
            """PSUM->SBUF evacuation on whichever engine is less loaded."""
            e = bal.pick(src.shape[-1])
            if e == "act":
                nc.scalar.copy(dst, src)
            else:
                nc.vector.tensor_copy(out=dst, in_=src)

        with tc.tile_pool(name="persist", bufs=1) as persist, \
             tc.tile_pool(name="xs", bufs=2) as xs, \
             tc.tile_pool(name="fpsum", bufs=2, space="PSUM") as fpsum, \
             tc.tile_pool(name="spsum", bufs=4, space="PSUM") as spool, \
             tc.tile_pool(name="opsum", bufs=2, space="PSUM") as opool, \
             tc.tile_pool(name="ptp", bufs=8) as ptp, \
             tc.tile_pool(name="recp", bufs=2) as recp, \
             tc.tile_pool(name="onrm", bufs=2) as onrm, \
             tc.tile_pool(name="yout", bufs=3) as yout:
            b1_sb = persist.tile([P, 12], F32, tag="b1", name="b1_sb")
            w2_sb = persist.tile([P, HL * D // P, C], BF16, tag="w2", name="w2_sb")
            w1_sb = persist.tile([P, KT_C, M1], BF16, tag="w1", name="w1_sb")

            # per-chunk persistent activations (bf16)
            QTc = [[persist.tile([P, IB], BF16, tag=f"QT{pr}_{tc_}",
                                 name=f"QT{pr}_{tc_}")
                    for tc_ in range(NIC)] for pr in range(HL // 2)]
            KTc = [[persist.tile([P, IB], BF16, tag=f"KT{pr}_{tc_}",
                                 name=f"KT{pr}_{tc_}")
                    for tc_ in range(NIC)] for pr in range(HL // 2)]
            V = [persist.tile([P, HL * (D + 1)], BF16, tag=f"V{tt}",
                              name=f"V{tt}")
                 for tt in range(NT)]
            OPc = [[persist.tile([P, IB], BF16, tag=f"OP{pr}_{tc_}",
                                 name=f"OP{pr}_{tc_}")
                    for tc_ in range(NIC)] for pr in range(HL // 2)]

            # V denominator columns = 1.0, written once on the idle Pool
            for tt in range(NT):
                nc.gpsimd.memset(
                    V[tt].rearrange("p (h x) -> p h x", x=D + 1)[:, :, D:D + 1],
                    1.0)
            if fake_exp == "skip":      # timing ablation: no exp at all
                fixed_pts = [persist.tile([P, G * IB], BF16, tag=f"fpt{i}",
                                          name=f"fpt{i}") for i in range(6)]
                for t_ in fixed_pts:
                    nc.gpsimd.memset(t_[:, :], 1.0)
            if "C" not in phases:       # ablation builds: OPc never written
                for pr in range(HL // 2):
                    for tc_ in range(NIC):
                        nc.gpsimd.memset(OPc[pr][tc_][:, :], 0.0)

            # ---------------- phase B thunks (QKV projection chunk) --------
            # Staggered K=64 half-chains: phase i interleaves chain[i-1]'s
            # high half with chain[i]'s low half on opposite PE row halves.
            def b_chunk_thunks(tc_, fill):
                xc_cell = {}

                def dma_xc(tc_=tc_):
                    xc = xs.tile([P, KT_C, IB], BF16, tag="xc", name="xc")
                    nc.sync.dma_start(out=xc[:, 0:4, :],
                                      in_=xT_r[:, 0:4, tc_ * IB:(tc_ + 1) * IB])
                    nc.scalar.dma_start(out=xc[:, 4:8, :],
                                        in_=xT_r[:, 4:8,
                                                 tc_ * IB:(tc_ + 1) * IB])
                    xc_cell["t"] = xc
                fill.add(dma_xc)

                def v_mm(ttl, k, st, sp_, cell):
                    if st:
                        cell["t"] = fpsum.tile([P, HL * D], F32,
                                               tag="fp", name="vp")
                    nc.tensor.matmul(
                        cell["t"][:, :],
                        lhsT=xc_cell["t"][:, k, ttl * P:(ttl + 1) * P],
                        rhs=w1_sb[:, k, 2 * HL * D:3 * HL * D],
                        start=st, stop=sp_)

                def q_mm(pr, qk, k, st, sp_, cell):
                    if st:
                        cell["t"] = fpsum.tile([P, IB], F32,
                                               tag="fp", name="qp")
                    nc.tensor.matmul(
                        cell["t"][:, :],
                        lhsT=w1_sb[:, k,
                                   qk * HL * D + pr * P:
                                   qk * HL * D + (pr + 1) * P],
                        rhs=xc_cell["t"][:, k, :],
                        start=st, stop=sp_)

                def v_evac(tt, cell):
                    copy_psum(
                        V[tt].rearrange(
                            "p (h x) -> p h x", x=D + 1)[:, :, 0:D],
                        cell["t"].rearrange("p (h x) -> p h x", x=D))

                def q_evac(pr, qk, cell):
                    dst = (QTc if qk == 0 else KTc)[pr][tc_]
                    if b1_zero:
                        copy_psum(dst[:, :], cell["t"][:, :])
                    else:
                        nc.vector.tensor_scalar_add(
                            dst[:, :], cell["t"][:, :],
                            b1_sb[:, 4 * qk + pr:4 * qk + pr + 1])

                chains = []
                for ttl in range(IB // P):
                    tt = tc_ * (IB // P) + ttl
                    chains.append({
                        "mm": (lambda k, st, sp_, cell, ttl=ttl:
                               v_mm(ttl, k, st, sp_, cell)),
                        "evac": (lambda cell, tt=tt: v_evac(tt, cell)),
                        "cell": {}})
                for pr in range(HL // 2):
                    for qk in range(2):
                        chains.append({
                            "mm": (lambda k, st, sp_, cell, pr=pr, qk=qk:
                                   q_mm(pr, qk, k, st, sp_, cell)),
                            "evac": (lambda cell, pr=pr, qk=qk:
                                     q_evac(pr, qk, cell)),
                            "cell": {}})

                for ch in chains:
                    for k in range(KT_C):
                        def thunk(k=k, ch=ch):
                            ch["mm"](k, k == 0, k == KT_C - 1, ch["cell"])
                        fill.add(thunk)
                    fill.add(lambda ch=ch: ch["evac"](ch["cell"]))

            # ---------------- phase D thunks (out projection chunk) --------
            def d_chunk_thunks(tc_, fill, tail=False):
                def y_evac(ttl, cc, cell):
                    tt = tc_ * (IB // P) + ttl
                    ysb = yout.tile([P, IB], BF16, tag="ysb", name="ysb")
                    copy_psum(ysb[:, :], cell["t"][:, :])
                    (nc.scalar if tail else nc.sync).dma_start(
                        out=y[tt * P:(tt + 1) * P,
                              cc * IB:(cc + 1) * IB],
                        in_=ysb[:, :])

                for ttl in range(IB // P):
                    for cc in range(C // IB):
                        cell = {}
                        for pr in range(HL // 2):
                            def mm(pr=pr, ttl=ttl, cc=cc, cell=cell):
                                if pr == 0:
                                    cell["t"] = fpsum.tile(
                                        [P, IB], F32, tag="fp", name="yp")
                                nc.tensor.matmul(
                                    cell["t"][:, :],
                                    lhsT=OPc[pr][tc_][:,
                                                      ttl * P:(ttl + 1) * P],
                                    rhs=w2_sb[:, pr, cc * IB:(cc + 1) * IB],
                                    start=(pr == 0),
                                    stop=(pr == HL // 2 - 1))
                            fill.add(mm)
                        fill.add(lambda ttl=ttl, cc=cc, cell=cell:
                                 y_evac(ttl, cc, cell))

            # ---------------- fused pipeline over i-chunks -----------------
            fill = Filler()
            b_chunk_thunks(0, fill)
            fill.emit(1)        # xc(0) DMA first in the in-order SP queue
            nc.sync.dma_start(out=b1_sb[:, :],
                              in_=b1.rearrange("(m p) -> p m", p=P))
            for k in range(KT_C):
                eng = nc.sync if k % 2 == 0 else nc.scalar
                eng.dma_start(out=w1_sb[:, k, :],
                              in_=w1.rearrange("(k p) m -> p k m",
                                               p=P)[:, k, :])
            nc.scalar.dma_start(out=w2_sb[:, :, :],
                                in_=w2.rearrange("(k p) c -> p k c", p=P))
            fill.flush()

            for ic in range(NIC):
                fa, fb = Filler(), Filler()
                if "D" in phases and ic > 0:
                    d_chunk_thunks(ic - 1, fa)
                if ic + 1 < NIC:
                    b_chunk_thunks(ic + 1, fb)
                fb.emit(1)      # xc(ic+1) DMA prefetch
                while fa.thunks or fb.thunks:
                    if fa.thunks:
                        fill.add(fa.thunks.pop(0))
                    if fb.thunks:
                        fill.add(fb.thunks.pop(0))

                if "C" in phases:
                    njt = (ic + 1) * (IB // P)
                    for pr in range(HL // 2):
                        ops = [opool.tile([P, 4 * (D + 1)], F32, tag="op",
                                          name=f"o{h2}") for h2 in range(2)]
                        onorm = onrm.tile([P, IB], BF16, tag="on",
                                          name="onorm")
                        ngr = (njt + G - 1) // G
                        pends = []      # up to 2 (jts, pts) awaiting PV

                        def do_pv(jts, pts):
                            for h2 in range(2):
                                h = pr * 2 + h2
                                for jl, jt in enumerate(jts):
                                    for it in range(IB // P):
                                        git = ic * (IB // P) + it
                                        if jt > git:
                                            continue
                                        nc.tensor.matmul(
                                            ops[h2][:, it * (D + 1):
                                                    (it + 1) * (D + 1)],
                                            lhsT=pts[h2][:,
                                                         jl * IB + it * P:
                                                         jl * IB +
                                                         (it + 1) * P],
                                            rhs=V[jt][:, h * (D + 1):
                                                      (h + 1) * (D + 1)],
                                            start=(jt == 0 and it == 0),
                                            stop=(jt == njt - 1 and it == 3))
                                fill.emit(2)

                        for g in range(ngr):
                            jts = list(range(g * G, min((g + 1) * G, njt)))
                            nj = len(jts)
                            srels = [max(0, jt * P - ic * IB) for jt in jts]
                            sps = [spool.tile([P, G * IB], F32, tag="sp",
                                              name=f"sp{h2}")
                                   for h2 in range(2)]
                            # h2 innermost: consecutive K=64 S matmuls
                            # alternate PE row halves -> tile concurrency
                            for jl, jt in enumerate(jts):
                                srel = srels[jl]
                                for h2 in range(2):
                                    hs = h2 * D
                                    nc.tensor.matmul(
                                        sps[h2][:, jl * IB + srel:
                                                (jl + 1) * IB],
                                        lhsT=KTc[pr][jt // 4][
                                            hs:hs + D,
                                            (jt % 4) * P:(jt % 4 + 1) * P],
                                        rhs=QTc[pr][ic][
                                            hs:hs + D, srel:IB],
                                        start=True, stop=True)
                            if fake_exp == "skip":
                                pts = [fixed_pts[(2 * g + h2) % 6]
                                       for h2 in range(2)]
                            else:
                                pts = [ptp.tile([P, G * IB], BF16, tag="pt",
                                                name=f"pt{h2}")
                                       for h2 in range(2)]

                            def do_exp(h2, c0, c1):
                                cols = c1 - c0
                                if fake_exp:
                                    nc.vector.tensor_copy(
                                        out=pts[h2][:, c0:c1],
                                        in_=sps[h2][:, c0:c1])
                                    return
                                e = bal.pick(cols, exp=True)
                                if e == "act":
                                    nc.scalar.activation(
                                        pts[h2][:, c0:c1], sps[h2][:, c0:c1],
                                        EXPF, scale=float(SCALE))
                                else:
                                    nc.vector.tensor_scalar(
                                        out=pts[h2][:, c0:c1].bitcast(I16),
                                        in0=sps[h2][:, c0:c1],
                                        scalar1=A_SCH, scalar2=B_SCH,
                                        op0=MUL, op1=ADD)

                            for h2 in range(2):
                                if fake_exp == "skip":
                                    break
                                if not any(srels):
                                    do_exp(h2, 0, nj * IB)
                                else:
                                    for jl in range(nj):
                                        do_exp(h2, jl * IB + srels[jl],
                                               (jl + 1) * IB)
                            # causal mask for diagonal blocks (Pool)
                            for h2 in range(2):
                                if fake_exp == "skip":
                                    break
                                for jl, jt in enumerate(jts):
                                    rel = jt * P - ic * IB
                                    if rel >= 0:
                                        nc.gpsimd.affine_select(
                                            out=pts[h2][:, jl * IB + rel:
                                                        jl * IB + rel + P],
                                            in_=pts[h2][:, jl * IB + rel:
                                                        jl * IB + rel + P],
                                            pattern=[[1, P]],
                                            compare_op=mybir.AluOpType.is_ge,
                                            fill=0.0,
                                            base=0, channel_multiplier=-1)
                            fill.emit(3)
                            # software pipeline, depth 2: PV lags two groups
                            if len(pends) >= 3:
                                do_pv(*pends.pop(0))
                            pends.append((jts, pts))
                        for pend in pends:
                            do_pv(*pend)
                        # normalize per i-partition: O[i, d] * recip(den[i])
                        for h2 in range(2):
                            rcol = recp.tile([P, 4], F32, tag="rc",
                                             name="rcol")
                            den = ops[h2].rearrange("p (i x) -> p i x",
                                                    x=D + 1)[:, :, D:D + 1]
                            nc.vector.reciprocal_approx_fast(
                                out=rcol.rearrange("p (i x) -> p i x",
                                                   x=1)[:, :, :],
                                in_=den)
                            for it in range(IB // P):
                                nc.vector.tensor_scalar_mul(
                                    onorm[:, it * P + h2 * D:
                                          it * P + (h2 + 1) * D],
                                    ops[h2][:, it * (D + 1):it * (D + 1) + D],
                                    rcol[:, it:it + 1])
                            bal.charge("dve", 4 + 4 * D, n_instr=5)
                            fill.emit(2)
                        for it in range(IB // P):
                            nc.sync.dma_start_transpose(
                                out=OPc[pr][ic][:, it * P:(it + 1) * P],
                                in_=onorm[:, it * P:(it + 1) * P])
                        fill.emit(2)
                fill.flush()

            if "D" in phases:
                d_chunk_thunks(NIC - 1, fill, tail=True)
                fill.flush()
        if verbose:
            print(f"[build] balancer busy(ns): "
                  f"act={bal.busy['act']:.0f} dve={bal.busy['dve']:.0f} "
                  f"exp cols: act={bal.exp_cols['act']:.0f} "
                  f"dve={bal.exp_cols['dve']:.0f}")
    nc.compile()
    return nc


_NC_CACHE = {}


def _get_nc(key=()):
    if key not in _NC_CACHE:
        _NC_CACHE[key] = build_nc()
    return _NC_CACHE[key]


def shard_inputs(x, w_qkv, b_qkv, w_out):
    """Build the 8 per-core input maps (bf16 matmul inputs)."""
    import ml_dtypes
    bf = ml_dtypes.bfloat16
    x = np.asarray(x, dtype=np.float32)
    w_qkv = np.asarray(w_qkv, dtype=np.float32)
    b_qkv = np.asarray(b_qkv, dtype=np.float32)
    w_out = np.asarray(w_out, dtype=np.float32)
    in_maps = []
    for core in range(N_CORES):
        b, hg = core // 2, core % 2
        cs = hg * HL * D              # 512-wide contiguous head-group slice
        w1 = np.ascontiguousarray(np.concatenate(
            [w_qkv[:, cs:cs + HL * D],
             w_qkv[:, C + cs:C + cs + HL * D],
             w_qkv[:, 2 * C + cs:2 * C + cs + HL * D]], axis=1))
        b1 = np.ascontiguousarray(np.concatenate(
            [b_qkv[cs:cs + HL * D],
             b_qkv[C + cs:C + cs + HL * D],
             b_qkv[2 * C + cs:2 * C + cs + HL * D]]))
        in_maps.append({
            "xT": np.ascontiguousarray(x[b].T).astype(bf),
            "w1": w1.astype(bf),
            "b1": b1,
            "w2": np.ascontiguousarray(w_out[cs:cs + HL * D, :]).astype(bf),
        })
    return in_maps


def combine_outputs(results, b_qkv, w_out, b_out):
    """Sum per-batch partials from the two head-group cores + bias terms."""
    bias_vec = (np.asarray(b_qkv[2 * C:3 * C], dtype=np.float32) @
                np.asarray(w_out, dtype=np.float32) +
                np.asarray(b_out, dtype=np.float32))
    y = np.empty((B, T, C), dtype=np.float32)
    for b in range(B):
        y[b] = (np.asarray(results[2 * b]["y"], dtype=np.float32) +
                np.asarray(results[2 * b + 1]["y"], dtype=np.float32) +
                bias_vec)
    return y


def kernel(x, w_qkv, b_qkv, w_out, b_out, *, trace=False, _sink=None):
    from concourse.bass_utils import run_bass_kernel_spmd
    b1z = not np.any(np.asarray(b_qkv))
    nc = (_get_nc() if b1z
          else _NC_CACHE.setdefault("b1", build_nc(b1_zero=False)))
    in_maps = shard_inputs(x, w_qkv, b_qkv, w_out)
    res = run_bass_kernel_spmd(nc, in_maps, core_ids=list(range(N_CORES)),
                               trace=trace)
    if _sink is not None:
        _sink.append(res)
    return combine_outputs(res.results, b_qkv, w_out, b_out)
